# revision 2
# baseline (speedup 1.0000x reference)
"""Two-layer GCN (DGL GraphConv, norm='both') on 8 Trainium2 NeuronCores, v2.

Strategy (v2): dst-sharded across 8 cores; per layer the bf16 feature table is
AllGathered to every core's DRAM and each core dma_gathers its edges' source
rows (edges sorted by (chunk, src-bucket, dst-block), padded per (chunk,
bucket) only — tiles may span dst-block boundaries). The per-edge one-hot
scatter matrices are PRECOMPUTED ON HOST and streamed via plain DMA (removes
the VectorE is_equal wall); dst norms are folded into the layer-1 one-hot
values (bf16), so layer-1 PSUM accumulates the transposed agg [feat, dst]
directly and the flush chain is W1 -> relu -> W2 -> transpose with all
copies on the idle Scalar engine. Layer 2 uses {0,1} fp8 one-hots as the
matmul stationary and scales by nd at flush.
"""

import os
import sys

sys.path.insert(0, "/opt/trn_rl_repo")

import numpy as np

from concourse import bacc, mybir, tile
from concourse.bass_utils import run_bass_kernel_spmd

F32 = mybir.dt.float32
BF16 = mybir.dt.bfloat16
FP8 = mybir.dt.float8e4
I16 = mybir.dt.int16
NPBF16 = np.dtype(mybir.dt.np(BF16))
NPFP8 = np.dtype(mybir.dt.np(FP8))

N = 100000
E = 1600000
DIN = 128
DOUT = 64
NCORES = 8
DLOC = N // NCORES           # 12500 dst nodes per core
NBLK = (DLOC + 127) // 128   # 98 dst blocks per core (last has 84 rows)
LASTROWS = DLOC - (NBLK - 1) * 128
BUCKET = 32768               # int16 gather-index range
NBUCK = (N + BUCKET - 1) // BUCKET  # 4
BUCKET_ROWS = [min(BUCKET, N - b * BUCKET) for b in range(NBUCK)]
GB = int(os.environ.get("GCN_GB", "8"))   # dst blocks per chunk
NQ = (NBLK + GB - 1) // GB
SINGLE_PACKET = bool(int(os.environ.get("GCN_SP", "0")))
DEBUG = int(os.environ.get("GCN2_DEBUG", "0"))


def _plane16(arr):
    """int16 idx array -> [128, len//16] plane (16-part wrap, 8x replicated)."""
    return np.ascontiguousarray(np.tile(arr.reshape(-1, 16).T, (8, 1)))


def _pack_plane(v, fill=1.0):
    """[DLOC] -> [128, NBLK] with [p, k] = v[k*128+p]; pad rows get fill."""
    a = np.full(NBLK * 128, fill, np.float32)
    a[:DLOC] = v
    return np.ascontiguousarray(a.reshape(NBLK, 128).T)


def _prep(src, dst):
    src = np.asarray(src, np.int64)
    dst = np.asarray(dst, np.int64)
    core = dst // DLOC

    out_deg = np.bincount(src, minlength=N).astype(np.float32)
    in_deg = np.bincount(dst, minlength=N).astype(np.float32)
    ns = 1.0 / np.sqrt(np.maximum(out_deg, 1.0))
    nd = 1.0 / np.sqrt(np.maximum(in_deg, 1.0))

    per_core = []
    nseg = NQ * NBUCK
    counts = np.zeros((NCORES, nseg), np.int64)
    for c in range(NCORES):
        m = core == c
        s = src[m]
        d = dst[m] - c * DLOC
        blk = d >> 7
        buck = s >> 15
        q = blk // GB
        order = np.lexsort((blk, buck, q))
        s, d, blk, buck, q = s[order], d[order], blk[order], buck[order], q[order]
        seg = q * NBUCK + buck
        counts[c] = np.bincount(seg, minlength=nseg)
        per_core.append((s, d, seg))

    Lp = ((counts.max(0) + 127) // 128) * 128       # shared padded seg lens
    off = np.zeros(nseg + 1, np.int64)
    np.cumsum(Lp, out=off[1:])
    totl = int(off[-1])

    # per-core padded slot arrays: idx (int16) and local dst (-1 = pad)
    idx_arrs, dst_arrs = [], []
    for c in range(NCORES):
        s, d, seg = per_core[c]
        seg_start = np.zeros(nseg + 1, np.int64)
        np.cumsum(counts[c], out=seg_start[1:])
        ia = np.zeros(totl, np.int16)
        da = np.full(totl, -1, np.int64)
        for si in range(nseg):
            a, z = seg_start[si], seg_start[si + 1]
            o = off[si]
            ia[o:o + z - a] = (s[a:z] - (si % NBUCK) * BUCKET).astype(np.int16)
            da[o:o + z - a] = d[a:z]
        idx_arrs.append(ia)
        dst_arrs.append(da)

    # union schedule, BLOCK-MAJOR within each chunk: one pending PSUM
    # accumulation group per bank at a time (HW zero-region constraint).
    sched_si, sched_t, sched_k = [], [], []
    q_p0 = np.zeros(NQ + 1, np.int64)
    start_f, stop_f = [], []
    for q in range(NQ):
        gb = min(GB, NBLK - q * GB)
        pres = {}
        for b in range(NBUCK):
            si = q * NBUCK + b
            T = Lp[si] // 128
            pr = np.zeros((T, gb), bool)
            for c in range(NCORES):
                da = dst_arrs[c][off[si]:off[si] + Lp[si]]
                valid = da >= 0
                t = np.nonzero(valid)[0] >> 7
                kk = (da[valid] >> 7) - q * GB
                pr[t, kk] = True
            pres[b] = pr
        for j in range(gb):
            entries = []
            for b in range(NBUCK):
                for t in np.nonzero(pres[b][:, j])[0]:
                    entries.append((q * NBUCK + b, int(t), q * GB + j))
            assert entries, (q, j)
            for i, (si, t, k) in enumerate(entries):
                sched_si.append(si)
                sched_t.append(t)
                sched_k.append(k)
                start_f.append(i == 0)
                stop_f.append(i == len(entries) - 1)
        q_p0[q + 1] = len(sched_si)
    sched_si = np.array(sched_si, np.int64)
    sched_t = np.array(sched_t, np.int64)
    sched_k = np.array(sched_k, np.int64)
    start_f = np.array(start_f, bool)
    stop_f = np.array(stop_f, bool)
    P = len(sched_si)

    # map sched position for each edge: pmap[si, t, k_local] -> p
    Tmax = int(max(Lp)) // 128
    pmap = np.full((nseg, Tmax, GB), -1, np.int64)
    pmap[sched_si, sched_t, sched_k - (sched_si // NBUCK) * GB] = \
        np.arange(P, dtype=np.int64)

    # one-hot streams per core
    oh1_list, oh2_list = [], []
    for c in range(NCORES):
        da = dst_arrs[c]
        valid = np.nonzero(da >= 0)[0]
        dv = da[valid]
        si_v = np.searchsorted(off[1:], valid, side="right")
        t_v = (valid - off[si_v]) >> 7
        k_v = dv >> 7
        p_v = pmap[si_v, t_v, k_v - (si_v // NBUCK) * GB]
        assert (p_v >= 0).all()
        rows = valid & 127
        cols = p_v * 128 + (dv & 127)
        dloc_glob = c * DLOC + dv
        w1 = (ns[dloc_glob] * nd[dloc_glob]).astype(np.float32)
        oh1 = np.zeros((128, P * 128), np.float32)
        oh1[rows, cols] = w1
        oh1_list.append(oh1.astype(NPBF16))
        oh2 = np.zeros((128, P * 128), np.float32)
        oh2[rows, cols] = 1.0
        oh2_list.append(oh2.astype(NPFP8))

    idx_planes = [_plane16(ia) for ia in idx_arrs]

    sched = dict(
        Lp=Lp.tolist(), off=off.tolist(), totl=totl,
        si=sched_si.tolist(), t=sched_t.tolist(), k=sched_k.tolist(),
        p0=q_p0.tolist(), P=P,
        start=start_f.tolist(), stop=stop_f.tolist(),
    )
    ns_planes = [_pack_plane(ns[c * DLOC:(c + 1) * DLOC]) for c in range(NCORES)]
    nd_planes = [_pack_plane(nd[c * DLOC:(c + 1) * DLOC]) for c in range(NCORES)]
    return sched, idx_planes, oh1_list, oh2_list, ns_planes, nd_planes


def _build(sched):
    Lp = sched["Lp"]
    off = sched["off"]
    totl = sched["totl"]
    P = sched["P"]
    s_si, s_t, s_k = sched["si"], sched["t"], sched["k"]
    p0 = sched["p0"]
    s_start, s_stop = sched["start"], sched["stop"]

    nc = bacc.Bacc("TRN2", target_bir_lowering=False, num_devices=NCORES)

    feat = nc.dram_tensor("feat", [DLOC, DIN], F32, kind="ExternalInput")
    idx_all = nc.dram_tensor("idx_all", [128, totl // 16], I16,
                             kind="ExternalInput")
    oh1_all = nc.dram_tensor("oh1_all", [128, P * 128], BF16,
                             kind="ExternalInput")
    oh2_all = nc.dram_tensor("oh2_all", [128, P * 128], FP8,
                             kind="ExternalInput")
    nsp = nc.dram_tensor("nsp", [128, NBLK], F32, kind="ExternalInput")
    ndp = nc.dram_tensor("ndp", [128, NBLK], F32, kind="ExternalInput")
    w1 = nc.dram_tensor("w1", [DIN, DIN], BF16, kind="ExternalInput")
    w2 = nc.dram_tensor("w2", [DIN, DOUT], BF16, kind="ExternalInput")
    b1c = nc.dram_tensor("b1c", [128, 1], F32, kind="ExternalInput")
    b2b = nc.dram_tensor("b2b", [128, DOUT], F32, kind="ExternalInput")
    ident_in = nc.dram_tensor("ident", [128, 128], BF16, kind="ExternalInput")
    out = nc.dram_tensor("out", [DLOC, DOUT], F32, kind="ExternalOutput")

    ag1_in = nc.dram_tensor("ag1_in", [DLOC, DIN], BF16, kind="Internal")
    table1 = nc.dram_tensor("table1", [N, DIN], BF16, kind="Internal",
                            addr_space="Shared")
    ag2_in = nc.dram_tensor("ag2_in", [DLOC, DIN], BF16, kind="Internal")
    table2 = nc.dram_tensor("table2", [N, DIN], BF16, kind="Internal",
                            addr_space="Shared")

    nfull = (NBLK - 1) * 128
    feat_main = feat[0:nfull, :].rearrange("(k p) f -> p k f", p=128)
    feat_tail = feat[nfull:DLOC, :]

    RELU = mybir.ActivationFunctionType.Relu
    COPY = mybir.ActivationFunctionType.Copy

    with tile.TileContext(nc) as tc:
        with (
            tc.tile_pool(name="const", bufs=1) as cpool,
            tc.tile_pool(name="idxp", bufs=1) as ipool,
            tc.tile_pool(name="stage", bufs=2) as spool,
            tc.tile_pool(name="oh", bufs=1) as opool,
            tc.tile_pool(name="work", bufs=2) as wpool,
            tc.tile_pool(name="psum", bufs=1, space="PSUM") as pp,
        ):
            # ---- constants ----
            ident_t = cpool.tile([128, 128], BF16)
            nc.sync.dma_start(ident_t[:], ident_in[:])
            w1_t = cpool.tile([DIN, DIN], BF16)
            nc.sync.dma_start(w1_t[:], w1[:])
            w2_t = cpool.tile([DIN, DOUT], BF16)
            nc.sync.dma_start(w2_t[:], w2[:])
            b1_t = cpool.tile([128, 1], F32)
            nc.sync.dma_start(b1_t[:], b1c[:])
            b2_t = cpool.tile([128, DOUT], F32)
            nc.sync.dma_start(b2_t[:], b2b[:])
            ns_t = cpool.tile([128, NBLK], F32)
            nc.sync.dma_start(ns_t[:], nsp[:])
            nd_t = cpool.tile([128, NBLK], F32)
            nc.sync.dma_start(nd_t[:], ndp[:])

            idx_t = ipool.tile([128, totl // 16], I16)
            nc.sync.dma_start(idx_t[:], idx_all[:])

            # two h2-staging tiles with right half permanently zero
            h2s = []
            for i in range(2):
                h = cpool.tile([128, 128], BF16, tag=f"h2s{i}", name=f"h2s{i}")
                nc.vector.memset(h[:, DOUT:], 0.0)
                h2s.append(h)

            # ---- prescale: ag1_in rows = feature * ns, cast bf16 ----
            for k in range(NBLK):
                rows = 128 if k < NBLK - 1 else LASTROWS
                ft = wpool.tile([128, 128], F32, tag="pf", bufs=4)
                if k < NBLK - 1:
                    nc.sync.dma_start(ft[:], feat_main[:, k, :])
                else:
                    nc.sync.dma_start(ft[:rows, :], feat_tail)
                fb = wpool.tile([128, 128], BF16, tag="pb", bufs=4)
                nc.vector.tensor_scalar_mul(fb[:], ft[:], ns_t[:, k:k + 1])
                nc.sync.dma_start(ag1_in[k * 128:k * 128 + rows, :],
                                  fb[:rows, :])
            nc.gpsimd.collective_compute(
                "AllGather", mybir.AluOpType.bypass,
                replica_groups=[list(range(NCORES))],
                ins=[ag1_in[:]], outs=[table1[:]])

            # ---- edge pass over one layer ----
            def edge_pass(table, oh_all, oh_dtype, layer):
                for q in range(NQ):
                    qp0, qp1 = p0[q], p0[q + 1]
                    oh = opool.tile([128, (qp1 - qp0) * 128], oh_dtype,
                                    tag="oh")
                    nc.sync.dma_start(oh[:], oh_all[:, qp0 * 128:qp1 * 128])
                    stages = {}
                    for b in range(NBUCK):
                        si = q * NBUCK + b
                        L = Lp[si]
                        if L == 0:
                            continue
                        T = L // 128
                        st = spool.tile([128, T, 128], BF16, tag=f"st{b}")
                        nc.gpsimd.dma_gather(
                            st[:],
                            table[b * BUCKET:b * BUCKET + BUCKET_ROWS[b], :],
                            idx_t[:, off[si] // 16:(off[si] + L) // 16],
                            num_idxs=L, num_idxs_reg=L, elem_size=128,
                            single_packet=SINGLE_PACKET)
                        stages[b] = st
                    w = 128 if layer == 1 else DOUT
                    eps = [pp.tile([128, 4, w], F32, tag=f"e{i}",
                                   name=f"e{i}") for i in range(2)]
                    for p in range(qp0, qp1):
                        b = s_si[p] % NBUCK
                        t, k = s_t[p], s_k[p]
                        rel = (p - qp0) * 128
                        j = k % GB
                        if layer == 1:
                            nc.tensor.matmul(
                                eps[j // 4][:, j % 4, :],
                                stages[b][:, t, :],
                                oh[:, rel:rel + 128],
                                start=s_start[p], stop=s_stop[p])
                        else:
                            nc.tensor.matmul(
                                eps[j // 4][:, j % 4, :],
                                oh[:, rel:rel + 128],
                                stages[b][:, t, 0:DOUT],
                                start=s_start[p], stop=s_stop[p])
                    for k in range(q * GB, min((q + 1) * GB, NBLK)):
                        if layer == 1:
                            flush1(k, eps)
                        else:
                            flush2(k, eps)

            # ---- layer 1 flush: psum aggT [f,d] -> h2' rows into ag2_in ----
            def flush1(k, eps):
                rows = 128 if k < NBLK - 1 else LASTROWS
                j = k % GB
                s1 = wpool.tile([128, 128], BF16, tag="f1s1")
                nc.scalar.activation(s1[:], eps[j // 4][:, j % 4, :], COPY)
                py = pp.tile([128, 128], F32, tag=f"py{k % 2}")
                nc.tensor.matmul(py[:], w1_t[:], s1[:], start=True, stop=True)
                s2 = wpool.tile([128, 128], BF16, tag="f1s2")
                nc.scalar.activation(s2[:], py[:], RELU, bias=b1_t[:])
                ph = pp.tile([DOUT, 128], F32, tag=f"ph{k % 2}")
                nc.tensor.matmul(ph[:], w2_t[:], s2[:], start=True, stop=True)
                s3 = wpool.tile([DOUT, 128], BF16, tag="f1s3")
                nc.scalar.activation(s3[:], ph[:], COPY)
                pt = pp.tile([128, DOUT], BF16, tag=f"tp{k % 2}")
                nc.tensor.transpose(pt[:], s3[:], ident_t[:DOUT, :DOUT])
                h2 = h2s[k % 2]
                nc.scalar.activation(h2[:, 0:DOUT], pt[:], COPY)
                nc.sync.dma_start(ag2_in[k * 128:k * 128 + rows, :],
                                  h2[:rows, :])
                if DEBUG:
                    dbg = wpool.tile([128, DOUT], F32, tag="dbg")
                    if k == 0:
                        nc.scalar.activation(dbg[:], s1[:, 0:DOUT], COPY)
                    else:
                        nc.scalar.activation(dbg[:], pt[:], COPY)
                    nc.sync.dma_start(out[k * 128:k * 128 + rows, :],
                                      dbg[:rows, :])

            # ---- layer 2 flush: psum [d,64] * nd + b2 -> out ----
            def flush2(k, eps):
                rows = 128 if k < NBLK - 1 else LASTROWS
                j = k % GB
                o1 = wpool.tile([128, DOUT], F32, tag="f2a")
                nc.scalar.activation(o1[:], eps[j // 4][:, j % 4, :], COPY,
                                     scale=nd_t[:, k:k + 1])
                o2 = wpool.tile([128, DOUT], F32, tag="f2b")
                nc.vector.tensor_add(o2[:], o1[:], b2_t[:])
                nc.sync.dma_start(out[k * 128:k * 128 + rows, :], o2[:rows, :])

            edge_pass(table1, oh1_all, BF16, 1)

            if not DEBUG:
                nc.gpsimd.collective_compute(
                    "AllGather", mybir.AluOpType.bypass,
                    replica_groups=[list(range(NCORES))],
                    ins=[ag2_in[:]], outs=[table2[:]])

                edge_pass(table2, oh2_all, FP8, 2)

    nc.compile()
    return nc


_CACHE = {}


def kernel(feature, src, dst, W1, b1, W2, b2):
    feature = np.asarray(feature, np.float32)
    (sched, idx_planes, oh1_list, oh2_list,
     ns_planes, nd_planes) = _prep(src, dst)

    key = (sched["totl"], sched["P"], tuple(sched["p0"]))
    if key not in _CACHE:
        _CACHE[key] = _build(sched)
    nc = _CACHE[key]

    ident = np.eye(128, dtype=np.float32)
    b1c = np.asarray(b1, np.float32).reshape(128, 1)
    b2b = np.tile(np.asarray(b2, np.float32)[None, :], (128, 1))

    in_maps = []
    for c in range(NCORES):
        lo = c * DLOC
        in_maps.append({
            "feat": feature[lo:lo + DLOC],
            "idx_all": idx_planes[c],
            "oh1_all": oh1_list[c],
            "oh2_all": oh2_list[c],
            "nsp": ns_planes[c],
            "ndp": nd_planes[c],
            "w1": np.asarray(W1, np.float32).astype(NPBF16),
            "w2": np.asarray(W2, np.float32).astype(NPBF16),
            "b1c": b1c,
            "b2b": b2b,
            "ident": ident.astype(NPBF16),
        })
    res = run_bass_kernel_spmd(nc, in_maps, core_ids=list(range(NCORES)))
    global LAST_RESULT
    LAST_RESULT = res
    return np.concatenate([res.results[c]["out"] for c in range(NCORES)],
                          axis=0)


LAST_RESULT = None


# revision 4
# speedup vs baseline: 1.0398x; 1.0398x over previous
"""Two-layer GCN (DGL GraphConv, norm='both') on 8 Trainium2 NeuronCores, v2.

Strategy (v2): dst-sharded across 8 cores; per layer the bf16 feature table is
AllGathered to every core's DRAM and each core dma_gathers its edges' source
rows (edges sorted by (chunk, src-bucket, dst-block), padded per (chunk,
bucket) only — tiles may span dst-block boundaries). The per-edge one-hot
scatter matrices are PRECOMPUTED ON HOST and streamed via plain DMA (removes
the VectorE is_equal wall); dst norms are folded into the layer-1 one-hot
values (bf16), so layer-1 PSUM accumulates the transposed agg [feat, dst]
directly and the flush chain is W1 -> relu -> W2 -> transpose with all
copies on the idle Scalar engine. Layer 2 uses {0,1} fp8 one-hots as the
matmul stationary and scales by nd at flush.
"""

import os
import sys

sys.path.insert(0, "/opt/trn_rl_repo")

import numpy as np

from concourse import bacc, mybir, tile
from concourse.bass_utils import run_bass_kernel_spmd

F32 = mybir.dt.float32
BF16 = mybir.dt.bfloat16
FP8 = mybir.dt.float8e4
I16 = mybir.dt.int16
NPBF16 = np.dtype(mybir.dt.np(BF16))
NPFP8 = np.dtype(mybir.dt.np(FP8))

N = 100000
E = 1600000
DIN = 128
DOUT = 64
NCORES = 8
DLOC = N // NCORES           # 12500 dst nodes per core
NBLK = (DLOC + 127) // 128   # 98 dst blocks per core (last has 84 rows)
LASTROWS = DLOC - (NBLK - 1) * 128
BUCKET = 32768               # int16 gather-index range
NBUCK = (N + BUCKET - 1) // BUCKET  # 4
BUCKET_ROWS = [min(BUCKET, N - b * BUCKET) for b in range(NBUCK)]
GB = int(os.environ.get("GCN_GB", "8"))   # dst blocks per chunk
NQ = (NBLK + GB - 1) // GB
SINGLE_PACKET = bool(int(os.environ.get("GCN_SP", "0")))
DEBUG = int(os.environ.get("GCN2_DEBUG", "0"))


def _plane16(arr):
    """int16 idx array -> [128, len//16] plane (16-part wrap, 8x replicated)."""
    return np.ascontiguousarray(np.tile(arr.reshape(-1, 16).T, (8, 1)))


def _pack_plane(v, fill=1.0):
    """[DLOC] -> [128, NBLK] with [p, k] = v[k*128+p]; pad rows get fill."""
    a = np.full(NBLK * 128, fill, np.float32)
    a[:DLOC] = v
    return np.ascontiguousarray(a.reshape(NBLK, 128).T)


def _prep(src, dst):
    src = np.asarray(src, np.int64)
    dst = np.asarray(dst, np.int64)
    core = dst // DLOC

    out_deg = np.bincount(src, minlength=N).astype(np.float32)
    in_deg = np.bincount(dst, minlength=N).astype(np.float32)
    ns = 1.0 / np.sqrt(np.maximum(out_deg, 1.0))
    nd = 1.0 / np.sqrt(np.maximum(in_deg, 1.0))

    per_core = []
    nseg = NQ * NBUCK
    counts = np.zeros((NCORES, nseg), np.int64)
    for c in range(NCORES):
        m = core == c
        s = src[m]
        d = dst[m] - c * DLOC
        blk = d >> 7
        buck = s >> 15
        q = blk // GB
        order = np.lexsort((blk, buck, q))
        s, d, blk, buck, q = s[order], d[order], blk[order], buck[order], q[order]
        seg = q * NBUCK + buck
        counts[c] = np.bincount(seg, minlength=nseg)
        per_core.append((s, d, seg))

    Lp = ((counts.max(0) + 127) // 128) * 128       # shared padded seg lens
    off = np.zeros(nseg + 1, np.int64)
    np.cumsum(Lp, out=off[1:])
    totl = int(off[-1])

    # per-core padded slot arrays: idx (int16) and local dst (-1 = pad)
    idx_arrs, dst_arrs = [], []
    for c in range(NCORES):
        s, d, seg = per_core[c]
        seg_start = np.zeros(nseg + 1, np.int64)
        np.cumsum(counts[c], out=seg_start[1:])
        ia = np.zeros(totl, np.int16)
        da = np.full(totl, -1, np.int64)
        for si in range(nseg):
            a, z = seg_start[si], seg_start[si + 1]
            o = off[si]
            ia[o:o + z - a] = (s[a:z] - (si % NBUCK) * BUCKET).astype(np.int16)
            da[o:o + z - a] = d[a:z]
        idx_arrs.append(ia)
        dst_arrs.append(da)

    # union schedule, BLOCK-MAJOR within each chunk: one pending PSUM
    # accumulation group per bank at a time (HW zero-region constraint).
    sched_si, sched_t, sched_k = [], [], []
    q_p0 = np.zeros(NQ + 1, np.int64)
    start_f, stop_f = [], []
    for q in range(NQ):
        gb = min(GB, NBLK - q * GB)
        pres = {}
        for b in range(NBUCK):
            si = q * NBUCK + b
            T = Lp[si] // 128
            pr = np.zeros((T, gb), bool)
            for c in range(NCORES):
                da = dst_arrs[c][off[si]:off[si] + Lp[si]]
                valid = da >= 0
                t = np.nonzero(valid)[0] >> 7
                kk = (da[valid] >> 7) - q * GB
                pr[t, kk] = True
            pres[b] = pr
        for j in range(gb):
            entries = []
            for b in range(NBUCK):
                for t in np.nonzero(pres[b][:, j])[0]:
                    entries.append((q * NBUCK + b, int(t), q * GB + j))
            assert entries, (q, j)
            for i, (si, t, k) in enumerate(entries):
                sched_si.append(si)
                sched_t.append(t)
                sched_k.append(k)
                start_f.append(i == 0)
                stop_f.append(i == len(entries) - 1)
        q_p0[q + 1] = len(sched_si)
    sched_si = np.array(sched_si, np.int64)
    sched_t = np.array(sched_t, np.int64)
    sched_k = np.array(sched_k, np.int64)
    start_f = np.array(start_f, bool)
    stop_f = np.array(stop_f, bool)
    P = len(sched_si)

    # map sched position for each edge: pmap[si, t, k_local] -> p
    Tmax = int(max(Lp)) // 128
    pmap = np.full((nseg, Tmax, GB), -1, np.int64)
    pmap[sched_si, sched_t, sched_k - (sched_si // NBUCK) * GB] = \
        np.arange(P, dtype=np.int64)

    # one-hot streams per core
    oh1_list, oh2_list = [], []
    for c in range(NCORES):
        da = dst_arrs[c]
        valid = np.nonzero(da >= 0)[0]
        dv = da[valid]
        si_v = np.searchsorted(off[1:], valid, side="right")
        t_v = (valid - off[si_v]) >> 7
        k_v = dv >> 7
        p_v = pmap[si_v, t_v, k_v - (si_v // NBUCK) * GB]
        assert (p_v >= 0).all()
        rows = valid & 127
        cols = p_v * 128 + (dv & 127)
        dloc_glob = c * DLOC + dv
        s_glob = (idx_arrs[c][valid].astype(np.int64)
                  + (si_v % NBUCK) * BUCKET)
        w1 = (ns[s_glob] * ns[dloc_glob] * nd[dloc_glob]).astype(np.float32)
        oh1 = np.zeros((128, P * 128), np.float32)
        oh1[rows, cols] = w1
        oh1_list.append(oh1.astype(NPBF16))
        oh2 = np.zeros((128, P * 128), np.float32)
        oh2[rows, cols] = 1.0
        oh2_list.append(oh2.astype(NPFP8))

    idx_planes = [_plane16(ia) for ia in idx_arrs]

    sched = dict(
        Lp=Lp.tolist(), off=off.tolist(), totl=totl,
        si=sched_si.tolist(), t=sched_t.tolist(), k=sched_k.tolist(),
        p0=q_p0.tolist(), P=P,
        start=start_f.tolist(), stop=stop_f.tolist(),
    )
    ns_planes = [_pack_plane(ns[c * DLOC:(c + 1) * DLOC]) for c in range(NCORES)]
    nd_planes = [_pack_plane(nd[c * DLOC:(c + 1) * DLOC]) for c in range(NCORES)]
    return sched, idx_planes, oh1_list, oh2_list, ns_planes, nd_planes


def _build(sched):
    Lp = sched["Lp"]
    off = sched["off"]
    totl = sched["totl"]
    P = sched["P"]
    s_si, s_t, s_k = sched["si"], sched["t"], sched["k"]
    p0 = sched["p0"]
    s_start, s_stop = sched["start"], sched["stop"]

    nc = bacc.Bacc("TRN2", target_bir_lowering=False, num_devices=NCORES)

    idx_all = nc.dram_tensor("idx_all", [128, totl // 16], I16,
                             kind="ExternalInput")
    oh1_all = nc.dram_tensor("oh1_all", [128, P * 128], BF16,
                             kind="ExternalInput")
    oh2_all = nc.dram_tensor("oh2_all", [128, P * 128], FP8,
                             kind="ExternalInput")
    ndp = nc.dram_tensor("ndp", [128, NBLK], F32, kind="ExternalInput")
    w1 = nc.dram_tensor("w1", [DIN, DIN], BF16, kind="ExternalInput")
    w2 = nc.dram_tensor("w2", [DIN, DOUT], BF16, kind="ExternalInput")
    b1c = nc.dram_tensor("b1c", [128, 1], F32, kind="ExternalInput")
    b2b = nc.dram_tensor("b2b", [128, DOUT], F32, kind="ExternalInput")
    ident_in = nc.dram_tensor("ident", [128, 128], BF16, kind="ExternalInput")
    out = nc.dram_tensor("out", [DLOC, DOUT], F32, kind="ExternalOutput")

    table1 = nc.dram_tensor("table1", [N, DIN], BF16, kind="ExternalInput")
    ag2_in = nc.dram_tensor("ag2_in", [DLOC, DIN], BF16, kind="Internal")
    table2 = nc.dram_tensor("table2", [N, DIN], BF16, kind="Internal",
                            addr_space="Shared")

    RELU = mybir.ActivationFunctionType.Relu
    COPY = mybir.ActivationFunctionType.Copy

    with tile.TileContext(nc) as tc:
        with (
            tc.tile_pool(name="const", bufs=1) as cpool,
            tc.tile_pool(name="idxp", bufs=1) as ipool,
            tc.tile_pool(name="stage", bufs=2) as spool,
            tc.tile_pool(name="oh", bufs=1) as opool,
            tc.tile_pool(name="work", bufs=2) as wpool,
            tc.tile_pool(name="psum", bufs=1, space="PSUM") as pp,
        ):
            # ---- constants ----
            ident_t = cpool.tile([128, 128], BF16)
            nc.sync.dma_start(ident_t[:], ident_in[:])
            w1_t = cpool.tile([DIN, DIN], BF16)
            nc.sync.dma_start(w1_t[:], w1[:])
            w2_t = cpool.tile([DIN, DOUT], BF16)
            nc.sync.dma_start(w2_t[:], w2[:])
            b1_t = cpool.tile([128, 1], F32)
            nc.sync.dma_start(b1_t[:], b1c[:])
            b2_t = cpool.tile([128, DOUT], F32)
            nc.sync.dma_start(b2_t[:], b2b[:])
            nd_t = cpool.tile([128, NBLK], F32)
            nc.sync.dma_start(nd_t[:], ndp[:])

            idx_t = ipool.tile([128, totl // 16], I16)
            nc.sync.dma_start(idx_t[:], idx_all[:])

            # two h2-staging tiles with right half permanently zero
            h2s = []
            for i in range(2):
                h = cpool.tile([128, 128], BF16, tag=f"h2s{i}", name=f"h2s{i}")
                nc.vector.memset(h[:, DOUT:], 0.0)
                h2s.append(h)

            # ---- edge pass over one layer ----
            def edge_pass(table, oh_all, oh_dtype, layer):
                for q in range(NQ):
                    qp0, qp1 = p0[q], p0[q + 1]
                    oh = opool.tile([128, (qp1 - qp0) * 128], oh_dtype,
                                    tag="oh")
                    nc.sync.dma_start(oh[:], oh_all[:, qp0 * 128:qp1 * 128])
                    stages = {}
                    for b in range(NBUCK):
                        si = q * NBUCK + b
                        L = Lp[si]
                        if L == 0:
                            continue
                        T = L // 128
                        st = spool.tile([128, T, 128], BF16, tag=f"st{b}")
                        nc.gpsimd.dma_gather(
                            st[:],
                            table[b * BUCKET:b * BUCKET + BUCKET_ROWS[b], :],
                            idx_t[:, off[si] // 16:(off[si] + L) // 16],
                            num_idxs=L, num_idxs_reg=L, elem_size=128,
                            single_packet=SINGLE_PACKET)
                        stages[b] = st
                    w = 128 if layer == 1 else DOUT
                    eps = [pp.tile([128, 4, w], F32, tag=f"e{i}",
                                   name=f"e{i}") for i in range(2)]
                    for p in range(qp0, qp1):
                        b = s_si[p] % NBUCK
                        t, k = s_t[p], s_k[p]
                        rel = (p - qp0) * 128
                        j = k % GB
                        if layer == 1:
                            nc.tensor.matmul(
                                eps[j // 4][:, j % 4, :],
                                stages[b][:, t, :],
                                oh[:, rel:rel + 128],
                                start=s_start[p], stop=s_stop[p])
                        else:
                            nc.tensor.matmul(
                                eps[j // 4][:, j % 4, :],
                                oh[:, rel:rel + 128],
                                stages[b][:, t, 0:DOUT],
                                start=s_start[p], stop=s_stop[p])
                    for k in range(q * GB, min((q + 1) * GB, NBLK)):
                        if layer == 1:
                            flush1(k, eps)
                        else:
                            flush2(k, eps)

            # ---- layer 1 flush: psum aggT [f,d] -> h2' rows into ag2_in ----
            def flush1(k, eps):
                rows = 128 if k < NBLK - 1 else LASTROWS
                j = k % GB
                s1 = wpool.tile([128, 128], BF16, tag="f1s1")
                nc.scalar.activation(s1[:], eps[j // 4][:, j % 4, :], COPY)
                py = pp.tile([128, 128], F32, tag=f"py{k % 2}")
                nc.tensor.matmul(py[:], w1_t[:], s1[:], start=True, stop=True)
                s2 = wpool.tile([128, 128], BF16, tag="f1s2")
                nc.scalar.activation(s2[:], py[:], RELU, bias=b1_t[:])
                ph = pp.tile([DOUT, 128], F32, tag=f"ph{k % 2}")
                nc.tensor.matmul(ph[:], w2_t[:], s2[:], start=True, stop=True)
                s3 = wpool.tile([DOUT, 128], BF16, tag="f1s3")
                nc.scalar.activation(s3[:], ph[:], COPY)
                pt = pp.tile([128, DOUT], BF16, tag=f"tp{k % 2}")
                nc.tensor.transpose(pt[:], s3[:], ident_t[:DOUT, :DOUT])
                h2 = h2s[k % 2]
                nc.scalar.activation(h2[:, 0:DOUT], pt[:], COPY)
                nc.sync.dma_start(ag2_in[k * 128:k * 128 + rows, :],
                                  h2[:rows, :])
                if DEBUG:
                    dbg = wpool.tile([128, DOUT], F32, tag="dbg")
                    if k == 0:
                        nc.scalar.activation(dbg[:], s1[:, 0:DOUT], COPY)
                    else:
                        nc.scalar.activation(dbg[:], pt[:], COPY)
                    nc.sync.dma_start(out[k * 128:k * 128 + rows, :],
                                      dbg[:rows, :])

            # ---- layer 2 flush: psum [d,64] * nd + b2 -> out ----
            def flush2(k, eps):
                rows = 128 if k < NBLK - 1 else LASTROWS
                j = k % GB
                o1 = wpool.tile([128, DOUT], F32, tag="f2a")
                nc.scalar.activation(o1[:], eps[j // 4][:, j % 4, :], COPY,
                                     scale=nd_t[:, k:k + 1])
                o2 = wpool.tile([128, DOUT], F32, tag="f2b")
                nc.vector.tensor_add(o2[:], o1[:], b2_t[:])
                nc.sync.dma_start(out[k * 128:k * 128 + rows, :], o2[:rows, :])

            edge_pass(table1, oh1_all, BF16, 1)

            if not DEBUG:
                nc.gpsimd.collective_compute(
                    "AllGather", mybir.AluOpType.bypass,
                    replica_groups=[list(range(NCORES))],
                    ins=[ag2_in[:]], outs=[table2[:]])

                edge_pass(table2, oh2_all, FP8, 2)

    nc.compile()
    return nc


_CACHE = {}


def kernel(feature, src, dst, W1, b1, W2, b2):
    feature = np.asarray(feature, np.float32)
    (sched, idx_planes, oh1_list, oh2_list,
     ns_planes, nd_planes) = _prep(src, dst)

    key = (sched["totl"], sched["P"], tuple(sched["p0"]))
    if key not in _CACHE:
        _CACHE[key] = _build(sched)
    nc = _CACHE[key]

    ident = np.eye(128, dtype=np.float32)
    b1c = np.asarray(b1, np.float32).reshape(128, 1)
    b2b = np.tile(np.asarray(b2, np.float32)[None, :], (128, 1))

    table1_bf = np.ascontiguousarray(feature.astype(NPBF16))
    in_maps = []
    for c in range(NCORES):
        in_maps.append({
            "table1": table1_bf,
            "idx_all": idx_planes[c],
            "oh1_all": oh1_list[c],
            "oh2_all": oh2_list[c],
            "ndp": nd_planes[c],
            "w1": np.asarray(W1, np.float32).astype(NPBF16),
            "w2": np.asarray(W2, np.float32).astype(NPBF16),
            "b1c": b1c,
            "b2b": b2b,
            "ident": ident.astype(NPBF16),
        })
    res = run_bass_kernel_spmd(nc, in_maps, core_ids=list(range(NCORES)))
    global LAST_RESULT
    LAST_RESULT = res
    return np.concatenate([res.results[c]["out"] for c in range(NCORES)],
                          axis=0)


LAST_RESULT = None
